# revision 21
# baseline (speedup 1.0000x reference)
"""GQA decode attention kernel for Trainium2, tensor-parallel over 8 kv heads.

Contract: kernel(**inputs) takes FULL inputs (numpy), returns FULL output.
Shapes are hardcoded: x[8,16,4096], w_in[6144,4096], w_out[4096,4096],
k_cache[8,4096,8,128], v_cache[8,4096,8,128], offset=4080.

Per-core (core g owns kv head g, q heads 4g..4g+3):
  qkv = x @ w_in_g.T            -> [128, 768] (q 512 | k 128 | v 128)
  rope(q, k), scatter new k/v into cache tail (T=4096)
  scoresT[t, (r,s)] = kkT chunks.T @ qT    (PE, per batch)
  expS = exp(scores)            (ACT, no max-sub: |scores| < ~8)
  denom = ones.T @ expS         (PE accumulate)
  outT = vv.T @ expS            (PE accumulate) ; scaled by 1/denom
  partial = attn_out @ w_out[:, 512g:+512].T  -> [128, 4096]
Host sums the 8 partials.

Schedule: all input DMAs go through the sync queue in priority order
(x/w_in interleaved -> ropes -> per-batch kv halves with w_out slices
after batch 3), so the kv stream overlaps the qkv projection and the
attention loop runs DMA-paced.  PV/denom/normalize run with a 1-batch
lag behind scores/exp so the PE never idles waiting on the activation
engine (idle resets the PE's 2.4GHz p-state ramp).
"""

import os
import sys

for _p in ("/opt/trn_rl_repo", "/root/.axon_site/_ro/trn_rl_repo"):
    if os.path.isdir(_p) and _p not in sys.path:
        sys.path.insert(0, _p)

import numpy as np
import ml_dtypes

BF16 = ml_dtypes.bfloat16

B, S, E = 8, 16, 4096
HQ, HKV, HD = 32, 8, 128
R = HQ // HKV          # 4 q heads per kv head
T = 4096               # cache length == offset + S
OFFSET = 4080
NCORES = 8
ROPE_BASE = 10000.0
BS = B * S             # 128 rows
QF = R * HD            # 512 q features per core
KCH = E // 128         # 32 contraction chunks for qkv proj
TCH = T // 128         # 32 T chunks
HT = T // 2            # 2048 cols per kv half tile

_CACHED = {}


def _build_program():
    """Build the Bass program once (same program for all cores)."""
    from concourse import bacc, bass, masks, mybir
    from concourse import tile

    f32 = mybir.dt.float32
    bf16 = mybir.dt.bfloat16
    f16 = mybir.dt.float16
    ActExp = mybir.ActivationFunctionType.Exp

    nc = bacc.Bacc(
        "TRN2",
        target_bir_lowering=False,
        debug=False,
        enable_asserts=False,
        num_devices=NCORES,
    )

    # DRAM I/O (per-core shards, host pre-permuted so every DMA is a plain
    # [128, N] contiguous-per-partition transfer)
    xT_d = nc.dram_tensor("xT", [128, KCH * 128], bf16, kind="ExternalInput").ap()
    win_d = nc.dram_tensor("w_inT", [128, KCH * 768], bf16, kind="ExternalInput").ap()
    # w_out pre-permuted to [128(d of this core), (n=8, r=4, 512)]
    wout_d = nc.dram_tensor("w_outT", [128, 4 * E], bf16, kind="ExternalInput").ap()
    ropec_d = nc.dram_tensor("rope_c", [128, 640], f32, kind="ExternalInput").ap()
    ropes_d = nc.dram_tensor("rope_s", [128, 640], f32, kind="ExternalInput").ap()
    kT_d = nc.dram_tensor("kT", [B, 128, T], bf16, kind="ExternalInput").ap()
    v_d = nc.dram_tensor("vperm", [B, 128, T], bf16, kind="ExternalInput").ap()
    out_d = nc.dram_tensor("out", [BS, E], f16, kind="ExternalOutput").ap()

    with tile.TileContext(nc) as tc:
        from contextlib import ExitStack

        with ExitStack() as ctx:
            const = ctx.enter_context(tc.tile_pool(name="const", bufs=1))
            winp = ctx.enter_context(tc.tile_pool(name="winp", bufs=1))
            woutp = ctx.enter_context(tc.tile_pool(name="woutp", bufs=1))
            work = ctx.enter_context(tc.tile_pool(name="work", bufs=1))
            kpool = ctx.enter_context(tc.tile_pool(name="kpool", bufs=9))
            vpool = ctx.enter_context(tc.tile_pool(name="vpool", bufs=9))
            epool = ctx.enter_context(tc.tile_pool(name="epool", bufs=3))
            bcp = ctx.enter_context(tc.tile_pool(name="bcp", bufs=2))
            opool = ctx.enter_context(tc.tile_pool(name="opool", bufs=4))
            ps_big = ctx.enter_context(
                tc.tile_pool(name="ps_big", bufs=4, space="PSUM")
            )
            ps_out = ctx.enter_context(
                tc.tile_pool(name="ps_out", bufs=3, space="PSUM")
            )
            ps_sm = ctx.enter_context(tc.tile_pool(name="ps_sm", bufs=1, space="PSUM"))

            # ---- all input DMAs, one hardware queue (sync), priority order.
            # x/w_in pieces interleaved (8 contraction chunks per piece) so
            # the qkv projection starts as soon as piece 0 lands.
            x_p = [
                const.tile([128, 8 * 128], bf16, tag=f"x{j}", name=f"x{j}")
                for j in range(4)
            ]
            w_p = [
                winp.tile([128, 8 * 768], bf16, tag=f"w{j}", name=f"w{j}")
                for j in range(4)
            ]
            ropeC = const.tile([128, 640], f32, tag="ropeC")
            ropeS = const.tile([128, 640], f32, tag="ropeS")
            for j in range(4):
                nc.sync.dma_start(x_p[j][:], xT_d[:, j * 1024 : (j + 1) * 1024])
                nc.sync.dma_start(w_p[j][:], win_d[:, j * 6144 : (j + 1) * 6144])
            # rope tables are needed right after the q/k projection finishes
            nc.sync.dma_start(ropeC[:], ropec_d[:])
            nc.sync.dma_start(ropeS[:], ropes_d[:])

            # kv cache per batch, in halves for finer pipelining; w_out
            # slices dropped in after batch 3 (needed for half-0 out proj)
            kk_h = [[None, None] for _ in range(B)]
            vv_h = [[None, None] for _ in range(B)]
            w_outT = [None, None]
            for b in range(B):
                for h in range(2):
                    kk_h[b][h] = kpool.tile([128, HT], bf16, tag="kk", name=f"kk{b}_{h}")
                    nc.sync.dma_start(kk_h[b][h][:], kT_d[b][:, h * HT : (h + 1) * HT])
                for h in range(2):
                    vv_h[b][h] = vpool.tile([128, HT], bf16, tag="vv", name=f"vv{b}_{h}")
                    nc.sync.dma_start(vv_h[b][h][:], v_d[b][:, h * HT : (h + 1) * HT])
                if b == 3:
                    for h in range(2):
                        w_outT[h] = woutp.tile(
                            [128, 2 * E], bf16, tag=f"wo{h}", name=f"wo{h}"
                        )
                        nc.sync.dma_start(
                            w_outT[h][:], wout_d[:, h * 2 * E : (h + 1) * 2 * E]
                        )

            # ---- constants (vector/gpsimd engines; do not block the DMA queue)
            ident = const.tile([128, 128], f32, tag="ident")
            masks.make_identity(nc, ident[:])
            ones_col = const.tile([128, 1], bf16, tag="ones_col")
            nc.vector.memset(ones_col[:], 1.0)
            ones_row = const.tile([1, 128], f32, tag="ones_row")
            nc.vector.memset(ones_row[:], 1.0)

            # ---- phase 1: q|k projection in psum, piece-paced.
            # q accumulates in a [128, 512] bank, k in a [128, 128] region;
            # the v projection is deferred past the transposes (it only gates
            # the lagged PV stage, not the attention start).
            q_ps = ps_big.tile([128, 512], f32, tag="sc", name="q_ps")
            k_ps = ps_sm.tile([128, 128], f32, tag="sm", name="k_ps")
            for k in range(KCH):
                j, kk_ = k // 8, k % 8
                nc.tensor.matmul(
                    q_ps[:],
                    x_p[j][:, kk_ * 128 : (kk_ + 1) * 128],
                    w_p[j][:, kk_ * 768 : kk_ * 768 + 512],
                    start=(k == 0),
                    stop=(k == KCH - 1),
                )
                nc.tensor.matmul(
                    k_ps[:],
                    x_p[j][:, kk_ * 128 : (kk_ + 1) * 128],
                    w_p[j][:, kk_ * 768 + 512 : kk_ * 768 + 640],
                    start=(k == 0),
                    stop=(k == KCH - 1),
                )

            # ---- rope on q and k: out = t*C + rot(t)*S.  Work is spread
            # across engines to shorten the serial chain: rope-table staging
            # on gpsimd (absorbs the DMA-queue dependency TT ops can't
            # carry), rotate-half copies on scalar, multiplies on vector.
            rot = work.tile([128, 640], f32, tag="rot")
            rot4 = rot[:].rearrange("p (blk h j) -> p blk h j", blk=5, h=2)
            ps4 = q_ps[:].rearrange("p (blk h j) -> p blk h j", blk=4, h=2)
            nc.scalar.copy(rot4[:, 0:4, 0, :], ps4[:, :, 1, :])
            nc.scalar.copy(rot4[:, 0:4, 1, :], ps4[:, :, 0, :])
            nc.scalar.copy(rot[:, 576:640], k_ps[:, 0:64])
            nc.scalar.copy(rot[:, 512:576], k_ps[:, 64:128])
            ropeCs = work.tile([128, 640], f32, tag="ropeCs")
            nc.gpsimd.tensor_copy(ropeCs[:], ropeC[:])
            ropeSs = work.tile([128, 640], f32, tag="ropeSs")
            nc.gpsimd.tensor_copy(ropeSs[:], ropeS[:])
            roped = work.tile([128, 640], f32, tag="roped")
            nc.vector.tensor_mul(roped[:, 0:512], q_ps[:], ropeCs[:, 0:512])
            nc.vector.tensor_mul(roped[:, 512:640], k_ps[:], ropeCs[:, 512:640])
            t2 = work.tile([128, 640], f32, tag="t2")
            nc.vector.tensor_mul(t2[:], rot[:], ropeSs[:])
            nc.vector.tensor_add(roped[:], roped[:], t2[:])

            # ---- transpose q heads -> q_allT [128(d), (b r s)] bf16
            q_allT = work.tile([128, B * R * S], bf16, tag="q_allT")
            qv = q_allT[:].rearrange("p (b r s) -> p b r s", b=B, r=R)
            for r in range(R):
                tp = ps_sm.tile([128, 128], f32, tag="sm")
                nc.tensor.transpose(tp[:], roped[:, r * 128 : (r + 1) * 128], ident[:])
                nc.vector.tensor_copy(
                    qv[:, :, r, :], tp[:].rearrange("p (b s) -> p b s", b=B)
                )
            # transpose new k -> kT_sb [128(d), (b s)] bf16
            kT_sb = work.tile([128, 128], bf16, tag="kT_sb")
            tpk = ps_sm.tile([128, 128], f32, tag="sm")
            nc.tensor.transpose(tpk[:], roped[:, 512:640], ident[:])
            nc.vector.tensor_copy(kT_sb[:], tpk[:])

            v_sb = work.tile([128, 128], bf16, tag="v_sb")

            def v_projection():
                # deferred v projection -> v_sb [128(bs), 128(d)] bf16, used
                # to patch the v cache tail per batch (patch is an SBUF->SBUF
                # DMA: compute engines need quadrant-aligned partition starts,
                # DMA does not); emitted after batch-0 scores so it does not
                # delay the attention start (it only gates the lagged PV)
                v_ps = ps_sm.tile([128, 128], f32, tag="sm", name="v_ps")
                for k in range(KCH):
                    j, kk_ = k // 8, k % 8
                    nc.tensor.matmul(
                        v_ps[:],
                        x_p[j][:, kk_ * 128 : (kk_ + 1) * 128],
                        w_p[j][:, kk_ * 768 + 640 : (kk_ + 1) * 768],
                        start=(k == 0),
                        stop=(k == KCH - 1),
                    )
                nc.vector.tensor_copy(v_sb[:], v_ps[:])

            # attention outputs per head, [128(d), (b s)] bf16, split in two
            # batch-halves so half-0's out-projection overlaps batches 4-7
            attn_halfT = [
                [
                    work.tile([128, 64], bf16, tag=f"attn{h}{r}", name=f"attn{h}{r}")
                    for r in range(R)
                ]
                for h in range(2)
            ]

            expS = [None] * B
            state = [None] * B  # (outT_ps, recip) per batch

            def emit_out_slice(h, n):
                part_ps = ps_out.tile([64, 512], f32, tag="po", name="part_ps")
                for r in range(R):
                    nc.tensor.matmul(
                        part_ps[:],
                        attn_halfT[h][r][:],
                        w_outT[n // 4][
                            :, (n % 4) * 2048 + r * 512 : (n % 4) * 2048 + (r + 1) * 512
                        ],
                        start=(r == 0),
                        stop=(r == R - 1),
                    )
                out_sb = opool.tile([64, 512], f16, tag="out_sb", name="out_sb")
                # alternate the psum->sbuf casts between vector and scalar so
                # the PE's out-proj stream never waits on a cast
                if n % 2 == 0:
                    nc.vector.tensor_copy(out_sb[:], part_ps[:])
                else:
                    nc.scalar.copy(out_sb[:], part_ps[:])
                nc.gpsimd.dma_start(
                    out_d[h * 64 : (h + 1) * 64, n * 512 : (n + 1) * 512],
                    out_sb[:],
                )

            # ---- phase 2: attention, PV/normalize lagging scores/exp by two
            # batches so the PE stream never waits on the activation engine
            for b in range(B + 2):
                if b < B:
                    # patch stale tail keys/values with roped new ones (the
                    # b==0 v-patch must be issued after v_projection() below:
                    # the tile framework tracks dependencies in program
                    # order, so a read issued before the writer sees garbage)
                    nc.vector.tensor_copy(
                        kk_h[b][1][:, OFFSET - HT : HT], kT_sb[:, b * S : (b + 1) * S]
                    )
                    if b > 0:
                        nc.gpsimd.dma_start(
                            vv_h[b][1][112:128, 15 * 128 : 16 * 128],
                            v_sb[b * S : (b + 1) * S, :],
                        )
                    expS[b] = epool.tile([128, TCH * 64], bf16, tag="expS", name=f"e{b}")
                    # scores in 4 one-bank psum quarters (ring of 4): the PE
                    # can run up to 3 quarters ahead of the exp ACTs instead
                    # of serializing on a 2-deep ring
                    for qtr in range(4):
                        sc = ps_big.tile([128, 512], f32, tag="sc", name=f"sc{b}_{qtr}")
                        for tt in range(8):
                            t = qtr * 8 + tt
                            nc.tensor.matmul(
                                sc[:, tt * 64 : (tt + 1) * 64],
                                kk_h[b][t // 16][:, (t % 16) * 128 : (t % 16 + 1) * 128],
                                q_allT[:, b * 64 : (b + 1) * 64],
                                start=True,
                                stop=True,
                            )
                        nc.scalar.activation(
                            expS[b][:, qtr * 512 : (qtr + 1) * 512],
                            sc[:],
                            ActExp,
                        )
                    if b == 0:
                        v_projection()
                        nc.gpsimd.dma_start(
                            vv_h[0][1][112:128, 15 * 128 : 16 * 128],
                            v_sb[0:S, :],
                        )
                if b > 1:
                    pb = b - 2
                    outT_ps = ps_out.tile([128, 64], f32, tag="po", name=f"o{pb}")
                    for t in range(TCH):
                        nc.tensor.matmul(
                            outT_ps[:],
                            vv_h[pb][t // 16][:, (t % 16) * 128 : (t % 16 + 1) * 128],
                            expS[pb][:, t * 64 : (t + 1) * 64],
                            start=(t == 0),
                            stop=(t == TCH - 1),
                        )
                    denom_ps = ps_sm.tile([1, 64], f32, tag="sm", name=f"d{pb}")
                    for t in range(TCH):
                        nc.tensor.matmul(
                            denom_ps[:],
                            ones_col[:],
                            expS[pb][:, t * 64 : (t + 1) * 64],
                            start=(t == 0),
                            stop=(t == TCH - 1),
                        )
                    recip = bcp.tile([1, 64], f32, tag="recip", name=f"r{pb}")
                    nc.vector.reciprocal(recip[:], denom_ps[:])
                    bc_ps = ps_sm.tile([128, 64], f32, tag="sm", name=f"bc{pb}")
                    nc.tensor.matmul(
                        bc_ps[:], ones_row[:], recip[:], start=True, stop=True
                    )
                    bc_sb = bcp.tile([128, 64], f32, tag="bc_sb", name=f"bc_sb{pb}")
                    nc.vector.tensor_copy(bc_sb[:], bc_ps[:])
                    for r in range(R):
                        nc.vector.tensor_mul(
                            attn_halfT[pb // 4][r][:, (pb % 4) * S : (pb % 4 + 1) * S],
                            outT_ps[:, r * S : (r + 1) * S],
                            bc_sb[:, r * S : (r + 1) * S],
                        )
                    # spread half-0's out-proj two slices per batch across
                    # batches 4-7 so it never blocks the scores/PV pipeline;
                    # half-1's slices can only run at the very end
                    if 3 <= pb <= 6:
                        emit_out_slice(0, 2 * (pb - 3))
                        emit_out_slice(0, 2 * (pb - 3) + 1)
                    elif pb == 7:
                        for n in range(8):
                            emit_out_slice(1, n)

    nc.compile()
    return nc


def _host_shards(x, w_in, w_out, k_cache, v_cache):
    """Per-core input dicts, pre-permuted for contiguous [128, N] DMAs."""
    x2 = np.ascontiguousarray(x.reshape(BS, E))
    xT_perm = (
        x2.T.reshape(KCH, 128, 128).transpose(1, 0, 2).reshape(128, KCH * 128)
    ).astype(BF16)

    # rope tables (identical on all cores); fold attn scale into q blocks
    inv_freq = 1.0 / (ROPE_BASE ** (np.arange(0, HD, 2, dtype=np.float64) / HD))
    pos = (OFFSET + np.arange(S)).astype(np.float64)
    ang = pos[:, None] * inv_freq[None, :]          # [S, 64]
    cos16 = np.cos(ang).astype(np.float32)
    sin16 = np.sin(ang).astype(np.float32)
    scale = np.float32(1.0 / np.sqrt(HD))
    C = np.zeros((128, 640), np.float32)
    Sn = np.zeros((128, 640), np.float32)
    srow = np.arange(128) % S                        # partition p=(b,s) -> s
    for blk in range(5):
        blk_scale = scale if blk < 4 else np.float32(1.0)
        C[:, blk * 128 : blk * 128 + 64] = cos16[srow] * blk_scale
        C[:, blk * 128 + 64 : blk * 128 + 128] = cos16[srow] * blk_scale
        Sn[:, blk * 128 : blk * 128 + 64] = -sin16[srow] * blk_scale
        Sn[:, blk * 128 + 64 : blk * 128 + 128] = sin16[srow] * blk_scale

    shards = []
    for g in range(NCORES):
        rows = np.concatenate(
            [
                w_in[QF * g : QF * (g + 1)],
                w_in[E + HD * g : E + HD * (g + 1)],
                w_in[E + HKV * HD + HD * g : E + HKV * HD + HD * (g + 1)],
            ],
            axis=0,
        )  # [768, 4096]
        w_inT_perm = (
            rows.T.reshape(KCH, 128, 768)
            .transpose(1, 0, 2)
            .reshape(128, KCH * 768)
        ).astype(BF16)
        # [128(d), (r, e)] -> [128(d), (n, r, 512)] so out-proj slice n is
        # contiguous and w_out can stream in two ordered pieces
        w_outT_perm = (
            w_out[:, QF * g : QF * (g + 1)]
            .T.reshape(4, 128, E)
            .transpose(1, 0, 2)
            .reshape(128, 4, 8, 512)
            .transpose(0, 2, 1, 3)
            .reshape(128, 4 * E)
        ).astype(BF16)
        kT = np.ascontiguousarray(
            k_cache[:, :, g, :].transpose(0, 2, 1)
        ).astype(BF16)  # [B, 128(d), T]
        vperm = np.ascontiguousarray(
            v_cache[:, :, g, :]
            .reshape(B, TCH, 128, HD)
            .transpose(0, 2, 1, 3)
            .reshape(B, 128, T)
        ).astype(BF16)  # [B, 128(t_in), (chunk d)]
        shards.append(
            {
                "xT": xT_perm,
                "w_inT": w_inT_perm,
                "w_outT": w_outT_perm,
                "rope_c": C,
                "rope_s": Sn,
                "kT": kT,
                "vperm": vperm,
            }
        )
    return shards


def _get_nc():
    if "nc" not in _CACHED:
        _CACHED["nc"] = _build_program()
    return _CACHED["nc"]


def run_on_hw(in_maps, trace=False, **kw):
    from concourse import bass_utils

    nc = _get_nc()
    return bass_utils.run_bass_kernel_spmd(
        nc, in_maps, core_ids=list(range(NCORES)), trace=trace, **kw
    )


def kernel(x, w_in, w_out, k_cache, v_cache, offset):
    assert int(offset) == OFFSET and x.shape == (B, S, E)
    shards = _host_shards(
        np.asarray(x, np.float32),
        np.asarray(w_in, np.float32),
        np.asarray(w_out, np.float32),
        np.asarray(k_cache, np.float32),
        np.asarray(v_cache, np.float32),
    )
    res = run_on_hw(shards)
    out = np.zeros((BS, E), np.float64)
    for g in range(NCORES):
        out += np.asarray(res.results[g]["out"], np.float64)
    return out.astype(np.float32).reshape(B, S, E)


# revision 25
# speedup vs baseline: 1.0044x; 1.0044x over previous
"""GQA decode attention kernel for Trainium2, tensor-parallel over 8 kv heads.

Contract: kernel(**inputs) takes FULL inputs (numpy), returns FULL output.
Shapes are hardcoded: x[8,16,4096], w_in[6144,4096], w_out[4096,4096],
k_cache[8,4096,8,128], v_cache[8,4096,8,128], offset=4080.

Per-core (core g owns kv head g, q heads 4g..4g+3):
  qkv = x @ w_in_g.T            -> [128, 768] (q 512 | k 128 | v 128)
  rope(q, k), scatter new k/v into cache tail (T=4096)
  scoresT[t, (r,s)] = kkT chunks.T @ qT    (PE, per batch)
  expS = exp(scores)            (ACT, no max-sub: |scores| < ~8)
  denom = ones.T @ expS         (PE accumulate)
  outT = vv.T @ expS            (PE accumulate) ; scaled by 1/denom
  partial = attn_out @ w_out[:, 512g:+512].T  -> [128, 4096]
Host sums the 8 partials.

Schedule: all input DMAs go through the sync queue in priority order
(x/w_in interleaved -> ropes -> per-batch kv halves with w_out slices
after batch 3), so the kv stream overlaps the qkv projection and the
attention loop runs DMA-paced.  PV/denom/normalize run with a 1-batch
lag behind scores/exp so the PE never idles waiting on the activation
engine (idle resets the PE's 2.4GHz p-state ramp).
"""

import os
import sys

for _p in ("/opt/trn_rl_repo", "/root/.axon_site/_ro/trn_rl_repo"):
    if os.path.isdir(_p) and _p not in sys.path:
        sys.path.insert(0, _p)

import numpy as np
import ml_dtypes

BF16 = ml_dtypes.bfloat16

B, S, E = 8, 16, 4096
HQ, HKV, HD = 32, 8, 128
R = HQ // HKV          # 4 q heads per kv head
T = 4096               # cache length == offset + S
OFFSET = 4080
NCORES = 8
ROPE_BASE = 10000.0
BS = B * S             # 128 rows
QF = R * HD            # 512 q features per core
KCH = E // 128         # 32 contraction chunks for qkv proj
TCH = T // 128         # 32 T chunks
HT = T // 2            # 2048 cols per kv half tile

_CACHED = {}


def _build_program():
    """Build the Bass program once (same program for all cores)."""
    from concourse import bacc, bass, masks, mybir
    from concourse import tile

    f32 = mybir.dt.float32
    bf16 = mybir.dt.bfloat16
    f16 = mybir.dt.float16
    ActExp = mybir.ActivationFunctionType.Exp

    nc = bacc.Bacc(
        "TRN2",
        target_bir_lowering=False,
        debug=False,
        enable_asserts=False,
        num_devices=NCORES,
    )

    # DRAM I/O (per-core shards, host pre-permuted so every DMA is a plain
    # [128, N] contiguous-per-partition transfer)
    xT_d = nc.dram_tensor("xT", [128, KCH * 128], bf16, kind="ExternalInput").ap()
    win_d = nc.dram_tensor("w_inT", [128, KCH * 768], bf16, kind="ExternalInput").ap()
    # w_out pre-permuted to [128(d of this core), (n=8, r=4, 512)]
    wout_d = nc.dram_tensor("w_outT", [128, 4 * E], bf16, kind="ExternalInput").ap()
    ropec_d = nc.dram_tensor("rope_c", [128, 640], f32, kind="ExternalInput").ap()
    ropes_d = nc.dram_tensor("rope_s", [128, 640], f32, kind="ExternalInput").ap()
    kT_d = nc.dram_tensor("kT", [B, 128, T], bf16, kind="ExternalInput").ap()
    v_d = nc.dram_tensor("vperm", [B, 128, T], bf16, kind="ExternalInput").ap()
    out_d = nc.dram_tensor("out", [BS, E], f16, kind="ExternalOutput").ap()

    with tile.TileContext(nc) as tc:
        from contextlib import ExitStack

        with ExitStack() as ctx:
            const = ctx.enter_context(tc.tile_pool(name="const", bufs=1))
            winp = ctx.enter_context(tc.tile_pool(name="winp", bufs=1))
            woutp = ctx.enter_context(tc.tile_pool(name="woutp", bufs=1))
            work = ctx.enter_context(tc.tile_pool(name="work", bufs=1))
            kpool = ctx.enter_context(tc.tile_pool(name="kpool", bufs=10))
            vpool = ctx.enter_context(tc.tile_pool(name="vpool", bufs=10))
            epool = ctx.enter_context(tc.tile_pool(name="epool", bufs=3))
            bcp = ctx.enter_context(tc.tile_pool(name="bcp", bufs=2))
            opool = ctx.enter_context(tc.tile_pool(name="opool", bufs=4))
            ps_big = ctx.enter_context(
                tc.tile_pool(name="ps_big", bufs=4, space="PSUM")
            )
            ps_out = ctx.enter_context(
                tc.tile_pool(name="ps_out", bufs=3, space="PSUM")
            )
            ps_sm = ctx.enter_context(tc.tile_pool(name="ps_sm", bufs=1, space="PSUM"))

            # ---- all input DMAs, one hardware queue (sync), priority order.
            # x/w_in pieces interleaved (8 contraction chunks per piece) so
            # the qkv projection starts as soon as piece 0 lands.
            x_p = [
                const.tile([128, 8 * 128], bf16, tag=f"x{j}", name=f"x{j}")
                for j in range(4)
            ]
            w_p = [
                winp.tile([128, 8 * 768], bf16, tag=f"w{j}", name=f"w{j}")
                for j in range(4)
            ]
            ropeC = const.tile([128, 640], f32, tag="ropeC")
            ropeS = const.tile([128, 640], f32, tag="ropeS")
            for j in range(4):
                nc.sync.dma_start(x_p[j][:], xT_d[:, j * 1024 : (j + 1) * 1024])
                nc.sync.dma_start(w_p[j][:], win_d[:, j * 6144 : (j + 1) * 6144])
            # rope tables are needed right after the q/k projection finishes
            nc.sync.dma_start(ropeC[:], ropec_d[:])
            nc.sync.dma_start(ropeS[:], ropes_d[:])

            # kv cache per batch, in halves for finer pipelining; w_out
            # slices dropped in after batch 3 (needed for half-0 out proj)
            kk_h = [[None, None] for _ in range(B)]
            vv_h = [[None, None] for _ in range(B)]
            w_outT = [None, None]
            for b in range(B):
                for h in range(2):
                    kk_h[b][h] = kpool.tile([128, HT], bf16, tag="kk", name=f"kk{b}_{h}")
                    nc.sync.dma_start(kk_h[b][h][:], kT_d[b][:, h * HT : (h + 1) * HT])
                for h in range(2):
                    vv_h[b][h] = vpool.tile([128, HT], bf16, tag="vv", name=f"vv{b}_{h}")
                    nc.sync.dma_start(vv_h[b][h][:], v_d[b][:, h * HT : (h + 1) * HT])
                if b == 3:
                    for h in range(2):
                        w_outT[h] = woutp.tile(
                            [128, 2 * E], bf16, tag=f"wo{h}", name=f"wo{h}"
                        )
                        nc.sync.dma_start(
                            w_outT[h][:], wout_d[:, h * 2 * E : (h + 1) * 2 * E]
                        )

            # ---- constants (vector/gpsimd engines; do not block the DMA queue)
            ident = const.tile([128, 128], f32, tag="ident")
            masks.make_identity(nc, ident[:])
            ones_col = const.tile([128, 1], bf16, tag="ones_col")
            nc.vector.memset(ones_col[:], 1.0)
            ones_row = const.tile([1, 128], f32, tag="ones_row")
            nc.vector.memset(ones_row[:], 1.0)

            # ---- phase 1: q|k projection in psum, piece-paced.
            # q accumulates in a [128, 512] bank, k in a [128, 128] region;
            # the v projection is deferred past the transposes (it only gates
            # the lagged PV stage, not the attention start).
            q_ps = ps_big.tile([128, 512], f32, tag="sc", name="q_ps")
            k_ps = ps_sm.tile([128, 128], f32, tag="sm", name="k_ps")
            for k in range(KCH):
                j, kk_ = k // 8, k % 8
                nc.tensor.matmul(
                    q_ps[:],
                    x_p[j][:, kk_ * 128 : (kk_ + 1) * 128],
                    w_p[j][:, kk_ * 768 : kk_ * 768 + 512],
                    start=(k == 0),
                    stop=(k == KCH - 1),
                )
                nc.tensor.matmul(
                    k_ps[:],
                    x_p[j][:, kk_ * 128 : (kk_ + 1) * 128],
                    w_p[j][:, kk_ * 768 + 512 : kk_ * 768 + 640],
                    start=(k == 0),
                    stop=(k == KCH - 1),
                )

            # ---- rope on q and k: out = t*C + rot(t)*S.  Work is spread
            # across engines to shorten the serial chain: rope-table staging
            # on gpsimd (absorbs the DMA-queue dependency TT ops can't
            # carry), rotate-half copies on scalar, multiplies on vector.
            rot = work.tile([128, 640], f32, tag="rot")
            rot4 = rot[:].rearrange("p (blk h j) -> p blk h j", blk=5, h=2)
            ps4 = q_ps[:].rearrange("p (blk h j) -> p blk h j", blk=4, h=2)
            nc.scalar.copy(rot4[:, 0:4, 0, :], ps4[:, :, 1, :])
            nc.scalar.copy(rot4[:, 0:4, 1, :], ps4[:, :, 0, :])
            nc.scalar.copy(rot[:, 576:640], k_ps[:, 0:64])
            nc.scalar.copy(rot[:, 512:576], k_ps[:, 64:128])
            ropeCs = work.tile([128, 640], f32, tag="ropeCs")
            nc.gpsimd.tensor_copy(ropeCs[:], ropeC[:])
            ropeSs = work.tile([128, 640], f32, tag="ropeSs")
            nc.gpsimd.tensor_copy(ropeSs[:], ropeS[:])
            roped = work.tile([128, 640], f32, tag="roped")
            nc.vector.tensor_mul(roped[:, 0:512], q_ps[:], ropeCs[:, 0:512])
            nc.vector.tensor_mul(roped[:, 512:640], k_ps[:], ropeCs[:, 512:640])
            t2 = work.tile([128, 640], f32, tag="t2")
            nc.vector.tensor_mul(t2[:], rot[:], ropeSs[:])
            nc.vector.tensor_add(roped[:], roped[:], t2[:])

            # ---- transpose q heads -> q_allT [128(d), (b r s)] bf16
            q_allT = work.tile([128, B * R * S], bf16, tag="q_allT")
            qv = q_allT[:].rearrange("p (b r s) -> p b r s", b=B, r=R)
            for r in range(R):
                tp = ps_out.tile([128, 128], f32, tag="po")
                nc.tensor.transpose(tp[:], roped[:, r * 128 : (r + 1) * 128], ident[:])
                nc.vector.tensor_copy(
                    qv[:, :, r, :], tp[:].rearrange("p (b s) -> p b s", b=B)
                )
            # transpose new k -> kT_sb [128(d), (b s)] bf16
            kT_sb = work.tile([128, 128], bf16, tag="kT_sb")
            tpk = ps_out.tile([128, 128], f32, tag="po")
            nc.tensor.transpose(tpk[:], roped[:, 512:640], ident[:])
            nc.vector.tensor_copy(kT_sb[:], tpk[:])

            v_sb = work.tile([128, 128], bf16, tag="v_sb")

            def v_projection():
                # deferred v projection -> v_sb [128(bs), 128(d)] bf16, used
                # to patch the v cache tail per batch (patch is an SBUF->SBUF
                # DMA: compute engines need quadrant-aligned partition starts,
                # DMA does not); emitted after batch-0 scores so it does not
                # delay the attention start (it only gates the lagged PV)
                v_ps = ps_out.tile([128, 128], f32, tag="po", name="v_ps")
                for k in range(KCH):
                    j, kk_ = k // 8, k % 8
                    nc.tensor.matmul(
                        v_ps[:],
                        x_p[j][:, kk_ * 128 : (kk_ + 1) * 128],
                        w_p[j][:, kk_ * 768 + 640 : (kk_ + 1) * 768],
                        start=(k == 0),
                        stop=(k == KCH - 1),
                    )
                nc.vector.tensor_copy(v_sb[:], v_ps[:])

            # attention outputs per head, [128(d), (b s)] bf16, split in two
            # batch-halves so half-0's out-projection overlaps batches 4-7
            attn_halfT = [
                [
                    work.tile([128, 64], bf16, tag=f"attn{h}{r}", name=f"attn{h}{r}")
                    for r in range(R)
                ]
                for h in range(2)
            ]

            expS = [None] * B
            state = [None] * B  # (outT_ps, recip) per batch

            def emit_out_slice(h, n):
                part_ps = ps_out.tile([64, 512], f32, tag="po", name="part_ps")
                for r in range(R):
                    nc.tensor.matmul(
                        part_ps[:],
                        attn_halfT[h][r][:],
                        w_outT[n // 4][
                            :, (n % 4) * 2048 + r * 512 : (n % 4) * 2048 + (r + 1) * 512
                        ],
                        start=(r == 0),
                        stop=(r == R - 1),
                    )
                out_sb = opool.tile([64, 512], f16, tag="out_sb", name="out_sb")
                # split each psum->sbuf cast across vector and scalar so the
                # PE's out-proj stream never waits on a cast
                nc.vector.tensor_copy(out_sb[:, 0:256], part_ps[:, 0:256])
                nc.scalar.copy(out_sb[:, 256:512], part_ps[:, 256:512])
                nc.gpsimd.dma_start(
                    out_d[h * 64 : (h + 1) * 64, n * 512 : (n + 1) * 512],
                    out_sb[:],
                )

            # ---- phase 2: attention, PV/normalize lagging scores/exp by two
            # batches so the PE stream never waits on the activation engine
            for b in range(B + 2):
                if b < B:
                    # patch stale tail keys/values with roped new ones (the
                    # b==0 v-patch must be issued after v_projection() below:
                    # the tile framework tracks dependencies in program
                    # order, so a read issued before the writer sees garbage)
                    nc.vector.tensor_copy(
                        kk_h[b][1][:, OFFSET - HT : HT], kT_sb[:, b * S : (b + 1) * S]
                    )
                    if b > 0:
                        nc.gpsimd.dma_start(
                            vv_h[b][1][112:128, 15 * 128 : 16 * 128],
                            v_sb[b * S : (b + 1) * S, :],
                        )
                    expS[b] = epool.tile([128, TCH * 64], bf16, tag="expS", name=f"e{b}")
                    # scores in 4 one-bank psum quarters (ring of 4): the PE
                    # can run up to 3 quarters ahead of the exp ACTs instead
                    # of serializing on a 2-deep ring
                    for qtr in range(4):
                        sc = ps_big.tile([128, 512], f32, tag="sc", name=f"sc{b}_{qtr}")
                        for tt in range(8):
                            t = qtr * 8 + tt
                            nc.tensor.matmul(
                                sc[:, tt * 64 : (tt + 1) * 64],
                                kk_h[b][t // 16][:, (t % 16) * 128 : (t % 16 + 1) * 128],
                                q_allT[:, b * 64 : (b + 1) * 64],
                                start=True,
                                stop=True,
                            )
                        nc.scalar.activation(
                            expS[b][:, qtr * 512 : (qtr + 1) * 512],
                            sc[:],
                            ActExp,
                        )
                    if b == 0:
                        v_projection()
                        nc.gpsimd.dma_start(
                            vv_h[0][1][112:128, 15 * 128 : 16 * 128],
                            v_sb[0:S, :],
                        )
                if b > 1:
                    pb = b - 2
                    outT_ps = ps_out.tile([128, 64], f32, tag="po", name=f"o{pb}")
                    for t in range(TCH):
                        nc.tensor.matmul(
                            outT_ps[:],
                            vv_h[pb][t // 16][:, (t % 16) * 128 : (t % 16 + 1) * 128],
                            expS[pb][:, t * 64 : (t + 1) * 64],
                            start=(t == 0),
                            stop=(t == TCH - 1),
                        )
                    denom_ps = ps_sm.tile([1, 64], f32, tag="sm", name=f"d{pb}")
                    for t in range(TCH):
                        nc.tensor.matmul(
                            denom_ps[:],
                            ones_col[:],
                            expS[pb][:, t * 64 : (t + 1) * 64],
                            start=(t == 0),
                            stop=(t == TCH - 1),
                        )
                    recip = bcp.tile([1, 64], f32, tag="recip", name=f"r{pb}")
                    nc.vector.reciprocal(recip[:], denom_ps[:])
                    bc_ps = ps_sm.tile([128, 64], f32, tag="sm", name=f"bc{pb}")
                    nc.tensor.matmul(
                        bc_ps[:], ones_row[:], recip[:], start=True, stop=True
                    )
                    bc_sb = bcp.tile([128, 64], f32, tag="bc_sb", name=f"bc_sb{pb}")
                    nc.vector.tensor_copy(bc_sb[:], bc_ps[:])
                    for r in range(R):
                        nc.vector.tensor_mul(
                            attn_halfT[pb // 4][r][:, (pb % 4) * S : (pb % 4 + 1) * S],
                            outT_ps[:, r * S : (r + 1) * S],
                            bc_sb[:, r * S : (r + 1) * S],
                        )
                    # spread half-0's out-proj two slices per batch across
                    # batches 4-7 so it never blocks the scores/PV pipeline;
                    # half-1's slices can only run at the very end
                    if 3 <= pb <= 6:
                        emit_out_slice(0, 2 * (pb - 3))
                        emit_out_slice(0, 2 * (pb - 3) + 1)
                    elif pb == 7:
                        for n in range(8):
                            emit_out_slice(1, n)

    nc.compile()
    return nc


def _host_shards(x, w_in, w_out, k_cache, v_cache):
    """Per-core input dicts, pre-permuted for contiguous [128, N] DMAs."""
    x2 = np.ascontiguousarray(x.reshape(BS, E))
    xT_perm = (
        x2.T.reshape(KCH, 128, 128).transpose(1, 0, 2).reshape(128, KCH * 128)
    ).astype(BF16)

    # rope tables (identical on all cores); fold attn scale into q blocks
    inv_freq = 1.0 / (ROPE_BASE ** (np.arange(0, HD, 2, dtype=np.float64) / HD))
    pos = (OFFSET + np.arange(S)).astype(np.float64)
    ang = pos[:, None] * inv_freq[None, :]          # [S, 64]
    cos16 = np.cos(ang).astype(np.float32)
    sin16 = np.sin(ang).astype(np.float32)
    scale = np.float32(1.0 / np.sqrt(HD))
    C = np.zeros((128, 640), np.float32)
    Sn = np.zeros((128, 640), np.float32)
    srow = np.arange(128) % S                        # partition p=(b,s) -> s
    for blk in range(5):
        blk_scale = scale if blk < 4 else np.float32(1.0)
        C[:, blk * 128 : blk * 128 + 64] = cos16[srow] * blk_scale
        C[:, blk * 128 + 64 : blk * 128 + 128] = cos16[srow] * blk_scale
        Sn[:, blk * 128 : blk * 128 + 64] = -sin16[srow] * blk_scale
        Sn[:, blk * 128 + 64 : blk * 128 + 128] = sin16[srow] * blk_scale

    shards = []
    for g in range(NCORES):
        rows = np.concatenate(
            [
                w_in[QF * g : QF * (g + 1)],
                w_in[E + HD * g : E + HD * (g + 1)],
                w_in[E + HKV * HD + HD * g : E + HKV * HD + HD * (g + 1)],
            ],
            axis=0,
        )  # [768, 4096]
        w_inT_perm = (
            rows.T.reshape(KCH, 128, 768)
            .transpose(1, 0, 2)
            .reshape(128, KCH * 768)
        ).astype(BF16)
        # [128(d), (r, e)] -> [128(d), (n, r, 512)] so out-proj slice n is
        # contiguous and w_out can stream in two ordered pieces
        w_outT_perm = (
            w_out[:, QF * g : QF * (g + 1)]
            .T.reshape(4, 128, E)
            .transpose(1, 0, 2)
            .reshape(128, 4, 8, 512)
            .transpose(0, 2, 1, 3)
            .reshape(128, 4 * E)
        ).astype(BF16)
        kT = np.ascontiguousarray(
            k_cache[:, :, g, :].transpose(0, 2, 1)
        ).astype(BF16)  # [B, 128(d), T]
        vperm = np.ascontiguousarray(
            v_cache[:, :, g, :]
            .reshape(B, TCH, 128, HD)
            .transpose(0, 2, 1, 3)
            .reshape(B, 128, T)
        ).astype(BF16)  # [B, 128(t_in), (chunk d)]
        shards.append(
            {
                "xT": xT_perm,
                "w_inT": w_inT_perm,
                "w_outT": w_outT_perm,
                "rope_c": C,
                "rope_s": Sn,
                "kT": kT,
                "vperm": vperm,
            }
        )
    return shards


def _get_nc():
    if "nc" not in _CACHED:
        _CACHED["nc"] = _build_program()
    return _CACHED["nc"]


def run_on_hw(in_maps, trace=False, **kw):
    from concourse import bass_utils

    nc = _get_nc()
    return bass_utils.run_bass_kernel_spmd(
        nc, in_maps, core_ids=list(range(NCORES)), trace=trace, **kw
    )


def kernel(x, w_in, w_out, k_cache, v_cache, offset):
    assert int(offset) == OFFSET and x.shape == (B, S, E)
    shards = _host_shards(
        np.asarray(x, np.float32),
        np.asarray(w_in, np.float32),
        np.asarray(w_out, np.float32),
        np.asarray(k_cache, np.float32),
        np.asarray(v_cache, np.float32),
    )
    res = run_on_hw(shards)
    out = np.zeros((BS, E), np.float64)
    for g in range(NCORES):
        out += np.asarray(res.results[g]["out"], np.float64)
    return out.astype(np.float32).reshape(B, S, E)


# revision 29
# speedup vs baseline: 1.0332x; 1.0286x over previous
"""GQA decode attention kernel for Trainium2, tensor-parallel over 8 kv heads.

Contract: kernel(**inputs) takes FULL inputs (numpy), returns FULL output.
Shapes are hardcoded: x[8,16,4096], w_in[6144,4096], w_out[4096,4096],
k_cache[8,4096,8,128], v_cache[8,4096,8,128], offset=4080.

Per-core (core g owns kv head g, q heads 4g..4g+3):
  qkv = x @ w_in_g.T            -> [128, 768] (q 512 | k 128 | v 128)
  rope(q, k), scatter new k/v into cache tail (T=4096)
  scoresT[t, (r,s)] = kkT chunks.T @ qT    (PE, per batch)
  expS = exp(scores)            (ACT, no max-sub: |scores| < ~8)
  denom = ones.T @ expS         (PE accumulate)
  outT = vv.T @ expS            (PE accumulate) ; scaled by 1/denom
  partial = attn_out @ w_out[:, 512g:+512].T  -> [128, 4096]
Host sums the 8 partials.

Schedule: all input DMAs go through the sync queue in priority order
(x/w_in interleaved -> ropes -> per-batch kv halves with w_out slices
after batch 3), so the kv stream overlaps the qkv projection and the
attention loop runs DMA-paced.  PV/denom/normalize run with a 1-batch
lag behind scores/exp so the PE never idles waiting on the activation
engine (idle resets the PE's 2.4GHz p-state ramp).
"""

import os
import sys

for _p in ("/opt/trn_rl_repo", "/root/.axon_site/_ro/trn_rl_repo"):
    if os.path.isdir(_p) and _p not in sys.path:
        sys.path.insert(0, _p)

import numpy as np
import ml_dtypes

BF16 = ml_dtypes.bfloat16

B, S, E = 8, 16, 4096
HQ, HKV, HD = 32, 8, 128
R = HQ // HKV          # 4 q heads per kv head
T = 4096               # cache length == offset + S
OFFSET = 4080
NCORES = 8
ROPE_BASE = 10000.0
BS = B * S             # 128 rows
QF = R * HD            # 512 q features per core
KCH = E // 128         # 32 contraction chunks for qkv proj
TCH = T // 128         # 32 T chunks
HT = T // 2            # 2048 cols per kv half tile

_CACHED = {}


def _build_program():
    """Build the Bass program once (same program for all cores)."""
    from concourse import bacc, bass, masks, mybir
    from concourse import tile

    f32 = mybir.dt.float32
    bf16 = mybir.dt.bfloat16
    f16 = mybir.dt.float16
    ActExp = mybir.ActivationFunctionType.Exp

    nc = bacc.Bacc(
        "TRN2",
        target_bir_lowering=False,
        debug=False,
        enable_asserts=False,
        num_devices=NCORES,
    )

    # DRAM I/O (per-core shards, host pre-permuted so every DMA is a plain
    # [128, N] contiguous-per-partition transfer)
    xT_d = nc.dram_tensor("xT", [128, KCH * 128], bf16, kind="ExternalInput").ap()
    win_d = nc.dram_tensor("w_inT", [128, KCH * 768], bf16, kind="ExternalInput").ap()
    # w_out pre-permuted to [128(d of this core), (n=8, r=4, 512)]
    wout_d = nc.dram_tensor("w_outT", [128, 4 * E], bf16, kind="ExternalInput").ap()
    ropec_d = nc.dram_tensor("rope_c", [128, 640], f32, kind="ExternalInput").ap()
    ropes_d = nc.dram_tensor("rope_s", [128, 640], f32, kind="ExternalInput").ap()
    kT_d = nc.dram_tensor("kT", [B, 128, T], bf16, kind="ExternalInput").ap()
    v_d = nc.dram_tensor("vperm", [B, 128, T], bf16, kind="ExternalInput").ap()
    out_d = nc.dram_tensor("out", [BS, E], f16, kind="ExternalOutput").ap()

    with tile.TileContext(nc) as tc:
        from contextlib import ExitStack

        with ExitStack() as ctx:
            const = ctx.enter_context(tc.tile_pool(name="const", bufs=1))
            winp = ctx.enter_context(tc.tile_pool(name="winp", bufs=1))
            woutp = ctx.enter_context(tc.tile_pool(name="woutp", bufs=1))
            work = ctx.enter_context(tc.tile_pool(name="work", bufs=1))
            kpool = ctx.enter_context(tc.tile_pool(name="kpool", bufs=10))
            vpool = ctx.enter_context(tc.tile_pool(name="vpool", bufs=10))
            epool = ctx.enter_context(tc.tile_pool(name="epool", bufs=3))
            bcp = ctx.enter_context(tc.tile_pool(name="bcp", bufs=2))
            opool = ctx.enter_context(tc.tile_pool(name="opool", bufs=4))
            ps_big = ctx.enter_context(
                tc.tile_pool(name="ps_big", bufs=4, space="PSUM")
            )
            ps_out = ctx.enter_context(
                tc.tile_pool(name="ps_out", bufs=3, space="PSUM")
            )
            ps_sm = ctx.enter_context(tc.tile_pool(name="ps_sm", bufs=1, space="PSUM"))

            # ---- all input DMAs, one hardware queue (sync), priority order.
            # x/w_in pieces interleaved (8 contraction chunks per piece) so
            # the qkv projection starts as soon as piece 0 lands.
            x_p = [
                const.tile([128, 8 * 128], bf16, tag=f"x{j}", name=f"x{j}")
                for j in range(4)
            ]
            w_p = [
                winp.tile([128, 8 * 768], bf16, tag=f"w{j}", name=f"w{j}")
                for j in range(4)
            ]
            ropeC = const.tile([128, 640], f32, tag="ropeC")
            ropeS = const.tile([128, 640], f32, tag="ropeS")
            for j in range(4):
                nc.sync.dma_start(x_p[j][:], xT_d[:, j * 1024 : (j + 1) * 1024])
                nc.sync.dma_start(w_p[j][:], win_d[:, j * 6144 : (j + 1) * 6144])
            # rope tables are needed right after the q/k projection finishes
            nc.sync.dma_start(ropeC[:], ropec_d[:])
            nc.sync.dma_start(ropeS[:], ropes_d[:])

            # kv cache per batch, in halves for finer pipelining; w_out
            # slices dropped in after batch 3 (needed for half-0 out proj)
            kk_h = [[None, None] for _ in range(B)]
            vv_h = [[None, None] for _ in range(B)]
            w_outT = [None, None]
            for b in range(B):
                for h in range(2):
                    kk_h[b][h] = kpool.tile([128, HT], bf16, tag="kk", name=f"kk{b}_{h}")
                    nc.sync.dma_start(kk_h[b][h][:], kT_d[b][:, h * HT : (h + 1) * HT])
                for h in range(2):
                    vv_h[b][h] = vpool.tile([128, HT], bf16, tag="vv", name=f"vv{b}_{h}")
                    nc.sync.dma_start(vv_h[b][h][:], v_d[b][:, h * HT : (h + 1) * HT])
                if b == 3:
                    for h in range(2):
                        w_outT[h] = woutp.tile(
                            [128, 2 * E], bf16, tag=f"wo{h}", name=f"wo{h}"
                        )
                        nc.sync.dma_start(
                            w_outT[h][:], wout_d[:, h * 2 * E : (h + 1) * 2 * E]
                        )

            # ---- constants (vector/gpsimd engines; do not block the DMA queue)
            ident = const.tile([128, 128], f32, tag="ident")
            masks.make_identity(nc, ident[:])
            ones_col = const.tile([128, 1], bf16, tag="ones_col")
            nc.vector.memset(ones_col[:], 1.0)
            ones_row = const.tile([1, 128], f32, tag="ones_row")
            nc.vector.memset(ones_row[:], 1.0)

            # ---- phase 1: q|k projection in psum, piece-paced.
            # q accumulates in a [128, 512] bank, k in a [128, 128] region;
            # the v projection is deferred past the transposes (it only gates
            # the lagged PV stage, not the attention start).
            q_ps = ps_big.tile([128, 512], f32, tag="sc", name="q_ps")
            k_ps = ps_sm.tile([128, 128], f32, tag="sm", name="k_ps")
            for k in range(KCH):
                j, kk_ = k // 8, k % 8
                nc.tensor.matmul(
                    q_ps[:],
                    x_p[j][:, kk_ * 128 : (kk_ + 1) * 128],
                    w_p[j][:, kk_ * 768 : kk_ * 768 + 512],
                    start=(k == 0),
                    stop=(k == KCH - 1),
                )
                nc.tensor.matmul(
                    k_ps[:],
                    x_p[j][:, kk_ * 128 : (kk_ + 1) * 128],
                    w_p[j][:, kk_ * 768 + 512 : kk_ * 768 + 640],
                    start=(k == 0),
                    stop=(k == KCH - 1),
                )

            # ---- rope on q and k: out = t*C + rot(t)*S.  Work is spread
            # across engines to shorten the serial chain: rope-table staging
            # on gpsimd (absorbs the DMA-queue dependency TT ops can't
            # carry), rotate-half copies on scalar, multiplies on vector.
            rot = work.tile([128, 640], f32, tag="rot")
            rot4 = rot[:].rearrange("p (blk h j) -> p blk h j", blk=5, h=2)
            ps4 = q_ps[:].rearrange("p (blk h j) -> p blk h j", blk=4, h=2)
            nc.scalar.copy(rot4[:, 0:4, 0, :], ps4[:, :, 1, :])
            nc.scalar.copy(rot4[:, 0:4, 1, :], ps4[:, :, 0, :])
            nc.scalar.copy(rot[:, 576:640], k_ps[:, 0:64])
            nc.scalar.copy(rot[:, 512:576], k_ps[:, 64:128])
            ropeCs = work.tile([128, 640], f32, tag="ropeCs")
            nc.gpsimd.tensor_copy(ropeCs[:], ropeC[:])
            ropeSs = work.tile([128, 640], f32, tag="ropeSs")
            nc.gpsimd.tensor_copy(ropeSs[:], ropeS[:])
            roped = work.tile([128, 640], f32, tag="roped")
            nc.vector.tensor_mul(roped[:, 0:512], q_ps[:], ropeCs[:, 0:512])
            nc.vector.tensor_mul(roped[:, 512:640], k_ps[:], ropeCs[:, 512:640])
            t2 = work.tile([128, 640], f32, tag="t2")
            nc.vector.tensor_mul(t2[:], rot[:], ropeSs[:])
            nc.vector.tensor_add(roped[:], roped[:], t2[:])

            # ---- transpose q heads -> q_allT [128(d), (b r s)] bf16
            q_allT = work.tile([128, B * R * S], bf16, tag="q_allT")
            qv = q_allT[:].rearrange("p (b r s) -> p b r s", b=B, r=R)
            for r in range(R):
                tp = ps_out.tile([128, 128], f32, tag="po")
                nc.tensor.transpose(tp[:], roped[:, r * 128 : (r + 1) * 128], ident[:])
                nc.vector.tensor_copy(
                    qv[:, :, r, :], tp[:].rearrange("p (b s) -> p b s", b=B)
                )
            # transpose new k -> kT_sb [128(d), (b s)] bf16
            kT_sb = work.tile([128, 128], bf16, tag="kT_sb")
            tpk = ps_out.tile([128, 128], f32, tag="po")
            nc.tensor.transpose(tpk[:], roped[:, 512:640], ident[:])
            nc.vector.tensor_copy(kT_sb[:], tpk[:])

            v_sb = work.tile([128, 128], bf16, tag="v_sb")

            def v_projection():
                # deferred v projection -> v_sb [128(bs), 128(d)] bf16, used
                # to patch the v cache tail per batch (patch is an SBUF->SBUF
                # DMA: compute engines need quadrant-aligned partition starts,
                # DMA does not); emitted after batch-0 scores so it does not
                # delay the attention start (it only gates the lagged PV)
                v_ps = ps_out.tile([128, 128], f32, tag="po", name="v_ps")
                for k in range(KCH):
                    j, kk_ = k // 8, k % 8
                    nc.tensor.matmul(
                        v_ps[:],
                        x_p[j][:, kk_ * 128 : (kk_ + 1) * 128],
                        w_p[j][:, kk_ * 768 + 640 : (kk_ + 1) * 768],
                        start=(k == 0),
                        stop=(k == KCH - 1),
                    )
                nc.vector.tensor_copy(v_sb[:], v_ps[:])

            # attention outputs per half, [128(d), (r, b, s)] bf16: one tile
            # per batch-half so the normalize scale is a single strided TT op
            attn_halfT = [
                work.tile([128, R * 64], bf16, tag=f"attn{h}", name=f"attn{h}")
                for h in range(2)
            ]

            expS = [None] * B
            state = [None] * B  # (outT_ps, recip) per batch

            def emit_out_slice(h, n):
                part_ps = ps_out.tile([64, 512], f32, tag="po", name="part_ps")
                for r in range(R):
                    nc.tensor.matmul(
                        part_ps[:],
                        attn_halfT[h][:, r * 64 : (r + 1) * 64],
                        w_outT[n // 4][
                            :, (n % 4) * 2048 + r * 512 : (n % 4) * 2048 + (r + 1) * 512
                        ],
                        start=(r == 0),
                        stop=(r == R - 1),
                    )
                out_sb = opool.tile([64, 512], f16, tag="out_sb", name="out_sb")
                # half-0 casts must stay off the scalar engine: a cast queued
                # there sits in front of later exp ACTs (program order) and
                # head-of-line-blocks the whole attention pipeline.  In the
                # half-1 (tail) flush all exps are already behind us, so
                # splitting each cast across vector+scalar halves the latency.
                if h == 0:
                    nc.vector.tensor_copy(out_sb[:], part_ps[:])
                else:
                    nc.vector.tensor_copy(out_sb[:, 0:256], part_ps[:, 0:256])
                    nc.scalar.copy(out_sb[:, 256:512], part_ps[:, 256:512])
                nc.gpsimd.dma_start(
                    out_d[h * 64 : (h + 1) * 64, n * 512 : (n + 1) * 512],
                    out_sb[:],
                )

            # ---- phase 2: attention, PV/normalize lagging scores/exp by two
            # batches so the PE stream never waits on the activation engine
            for b in range(B + 2):
                if b < B:
                    # patch stale tail keys/values with roped new ones (the
                    # b==0 v-patch must be issued after v_projection() below:
                    # the tile framework tracks dependencies in program
                    # order, so a read issued before the writer sees garbage)
                    nc.vector.tensor_copy(
                        kk_h[b][1][:, OFFSET - HT : HT], kT_sb[:, b * S : (b + 1) * S]
                    )
                    if b > 0:
                        nc.gpsimd.dma_start(
                            vv_h[b][1][112:128, 15 * 128 : 16 * 128],
                            v_sb[b * S : (b + 1) * S, :],
                        )
                    expS[b] = epool.tile([128, TCH * 64], bf16, tag="expS", name=f"e{b}")
                    # scores in 4 one-bank psum quarters (ring of 4): the PE
                    # can run up to 3 quarters ahead of the exp ACTs instead
                    # of serializing on a 2-deep ring
                    for qtr in range(4):
                        sc = ps_big.tile([128, 512], f32, tag="sc", name=f"sc{b}_{qtr}")
                        for tt in range(8):
                            t = qtr * 8 + tt
                            nc.tensor.matmul(
                                sc[:, tt * 64 : (tt + 1) * 64],
                                kk_h[b][t // 16][:, (t % 16) * 128 : (t % 16 + 1) * 128],
                                q_allT[:, b * 64 : (b + 1) * 64],
                                start=True,
                                stop=True,
                            )
                        nc.scalar.activation(
                            expS[b][:, qtr * 512 : (qtr + 1) * 512],
                            sc[:],
                            ActExp,
                        )
                    if b == 0:
                        v_projection()
                        nc.gpsimd.dma_start(
                            vv_h[0][1][112:128, 15 * 128 : 16 * 128],
                            v_sb[0:S, :],
                        )
                if b > 1:
                    pb = b - 2
                    outT_ps = ps_out.tile([128, 64], f32, tag="po", name=f"o{pb}")
                    for t in range(TCH):
                        nc.tensor.matmul(
                            outT_ps[:],
                            vv_h[pb][t // 16][:, (t % 16) * 128 : (t % 16 + 1) * 128],
                            expS[pb][:, t * 64 : (t + 1) * 64],
                            start=(t == 0),
                            stop=(t == TCH - 1),
                        )
                    denom_ps = ps_sm.tile([1, 64], f32, tag="sm", name=f"d{pb}")
                    for t in range(TCH):
                        nc.tensor.matmul(
                            denom_ps[:],
                            ones_col[:],
                            expS[pb][:, t * 64 : (t + 1) * 64],
                            start=(t == 0),
                            stop=(t == TCH - 1),
                        )
                    recip = bcp.tile([1, 64], f32, tag="recip", name=f"r{pb}")
                    nc.vector.reciprocal(recip[:], denom_ps[:])
                    bc_ps = ps_sm.tile([128, 64], f32, tag="sm", name=f"bc{pb}")
                    nc.tensor.matmul(
                        bc_ps[:], ones_row[:], recip[:], start=True, stop=True
                    )
                    bc_sb = bcp.tile([128, 64], f32, tag="bc_sb", name=f"bc_sb{pb}")
                    nc.vector.tensor_copy(bc_sb[:], bc_ps[:])
                    # single strided scale: attn[(r, pb%4, s)] = outT[(r, s)] * bc[(r, s)]
                    attn_view = attn_halfT[pb // 4][:].rearrange(
                        "p (r b s) -> p r b s", r=R, b=4
                    )
                    nc.vector.tensor_mul(
                        attn_view[:, :, pb % 4, :],
                        outT_ps[:].rearrange("p (r s) -> p r s", r=R),
                        bc_sb[:].rearrange("p (r s) -> p r s", r=R),
                    )
                    # spread half-0's out-proj two slices per batch across
                    # batches 4-7 so it never blocks the scores/PV pipeline;
                    # half-1's slices can only run at the very end
                    if 3 <= pb <= 6:
                        emit_out_slice(0, 2 * (pb - 3))
                        emit_out_slice(0, 2 * (pb - 3) + 1)
                    elif pb == 7:
                        for n in range(8):
                            emit_out_slice(1, n)

    nc.compile()
    return nc


def _host_shards(x, w_in, w_out, k_cache, v_cache):
    """Per-core input dicts, pre-permuted for contiguous [128, N] DMAs."""
    x2 = np.ascontiguousarray(x.reshape(BS, E))
    xT_perm = (
        x2.T.reshape(KCH, 128, 128).transpose(1, 0, 2).reshape(128, KCH * 128)
    ).astype(BF16)

    # rope tables (identical on all cores); fold attn scale into q blocks
    inv_freq = 1.0 / (ROPE_BASE ** (np.arange(0, HD, 2, dtype=np.float64) / HD))
    pos = (OFFSET + np.arange(S)).astype(np.float64)
    ang = pos[:, None] * inv_freq[None, :]          # [S, 64]
    cos16 = np.cos(ang).astype(np.float32)
    sin16 = np.sin(ang).astype(np.float32)
    scale = np.float32(1.0 / np.sqrt(HD))
    C = np.zeros((128, 640), np.float32)
    Sn = np.zeros((128, 640), np.float32)
    srow = np.arange(128) % S                        # partition p=(b,s) -> s
    for blk in range(5):
        blk_scale = scale if blk < 4 else np.float32(1.0)
        C[:, blk * 128 : blk * 128 + 64] = cos16[srow] * blk_scale
        C[:, blk * 128 + 64 : blk * 128 + 128] = cos16[srow] * blk_scale
        Sn[:, blk * 128 : blk * 128 + 64] = -sin16[srow] * blk_scale
        Sn[:, blk * 128 + 64 : blk * 128 + 128] = sin16[srow] * blk_scale

    shards = []
    for g in range(NCORES):
        rows = np.concatenate(
            [
                w_in[QF * g : QF * (g + 1)],
                w_in[E + HD * g : E + HD * (g + 1)],
                w_in[E + HKV * HD + HD * g : E + HKV * HD + HD * (g + 1)],
            ],
            axis=0,
        )  # [768, 4096]
        w_inT_perm = (
            rows.T.reshape(KCH, 128, 768)
            .transpose(1, 0, 2)
            .reshape(128, KCH * 768)
        ).astype(BF16)
        # [128(d), (r, e)] -> [128(d), (n, r, 512)] so out-proj slice n is
        # contiguous and w_out can stream in two ordered pieces
        w_outT_perm = (
            w_out[:, QF * g : QF * (g + 1)]
            .T.reshape(4, 128, E)
            .transpose(1, 0, 2)
            .reshape(128, 4, 8, 512)
            .transpose(0, 2, 1, 3)
            .reshape(128, 4 * E)
        ).astype(BF16)
        kT = np.ascontiguousarray(
            k_cache[:, :, g, :].transpose(0, 2, 1)
        ).astype(BF16)  # [B, 128(d), T]
        vperm = np.ascontiguousarray(
            v_cache[:, :, g, :]
            .reshape(B, TCH, 128, HD)
            .transpose(0, 2, 1, 3)
            .reshape(B, 128, T)
        ).astype(BF16)  # [B, 128(t_in), (chunk d)]
        shards.append(
            {
                "xT": xT_perm,
                "w_inT": w_inT_perm,
                "w_outT": w_outT_perm,
                "rope_c": C,
                "rope_s": Sn,
                "kT": kT,
                "vperm": vperm,
            }
        )
    return shards


def _get_nc():
    if "nc" not in _CACHED:
        _CACHED["nc"] = _build_program()
    return _CACHED["nc"]


def run_on_hw(in_maps, trace=False, **kw):
    from concourse import bass_utils

    nc = _get_nc()
    return bass_utils.run_bass_kernel_spmd(
        nc, in_maps, core_ids=list(range(NCORES)), trace=trace, **kw
    )


def kernel(x, w_in, w_out, k_cache, v_cache, offset):
    assert int(offset) == OFFSET and x.shape == (B, S, E)
    shards = _host_shards(
        np.asarray(x, np.float32),
        np.asarray(w_in, np.float32),
        np.asarray(w_out, np.float32),
        np.asarray(k_cache, np.float32),
        np.asarray(v_cache, np.float32),
    )
    res = run_on_hw(shards)
    out = np.zeros((BS, E), np.float64)
    for g in range(NCORES):
        out += np.asarray(res.results[g]["out"], np.float64)
    return out.astype(np.float32).reshape(B, S, E)


# revision 31
# speedup vs baseline: 1.0381x; 1.0048x over previous
"""GQA decode attention kernel for Trainium2, tensor-parallel over 8 kv heads.

Contract: kernel(**inputs) takes FULL inputs (numpy), returns FULL output.
Shapes are hardcoded: x[8,16,4096], w_in[6144,4096], w_out[4096,4096],
k_cache[8,4096,8,128], v_cache[8,4096,8,128], offset=4080.

Per-core (core g owns kv head g, q heads 4g..4g+3):
  qkv = x @ w_in_g.T            -> [128, 768] (q 512 | k 128 | v 128)
  rope(q, k), scatter new k/v into cache tail (T=4096)
  scoresT[t, (r,s)] = kkT chunks.T @ qT    (PE, per batch)
  expS = exp(scores)            (ACT, no max-sub: |scores| < ~8)
  denom = ones.T @ expS         (PE accumulate)
  outT = vv.T @ expS            (PE accumulate) ; scaled by 1/denom
  partial = attn_out @ w_out[:, 512g:+512].T  -> [128, 4096]
Host sums the 8 partials.

Schedule: all input DMAs go through the sync queue in priority order
(x/w_in interleaved -> ropes -> per-batch kv halves with w_out slices
after batch 3), so the kv stream overlaps the qkv projection and the
attention loop runs DMA-paced.  PV/denom/normalize run with a 1-batch
lag behind scores/exp so the PE never idles waiting on the activation
engine (idle resets the PE's 2.4GHz p-state ramp).
"""

import os
import sys

for _p in ("/opt/trn_rl_repo", "/root/.axon_site/_ro/trn_rl_repo"):
    if os.path.isdir(_p) and _p not in sys.path:
        sys.path.insert(0, _p)

import numpy as np
import ml_dtypes

BF16 = ml_dtypes.bfloat16

B, S, E = 8, 16, 4096
HQ, HKV, HD = 32, 8, 128
R = HQ // HKV          # 4 q heads per kv head
T = 4096               # cache length == offset + S
OFFSET = 4080
NCORES = 8
ROPE_BASE = 10000.0
BS = B * S             # 128 rows
QF = R * HD            # 512 q features per core
KCH = E // 128         # 32 contraction chunks for qkv proj
TCH = T // 128         # 32 T chunks
HT = T // 2            # 2048 cols per kv half tile

_CACHED = {}


def _build_program():
    """Build the Bass program once (same program for all cores)."""
    from concourse import bacc, bass, masks, mybir
    from concourse import tile

    f32 = mybir.dt.float32
    bf16 = mybir.dt.bfloat16
    f16 = mybir.dt.float16
    ActExp = mybir.ActivationFunctionType.Exp

    nc = bacc.Bacc(
        "TRN2",
        target_bir_lowering=False,
        debug=False,
        enable_asserts=False,
        num_devices=NCORES,
    )

    # DRAM I/O (per-core shards, host pre-permuted so every DMA is a plain
    # [128, N] contiguous-per-partition transfer)
    xT_d = nc.dram_tensor("xT", [128, KCH * 128], bf16, kind="ExternalInput").ap()
    win_d = nc.dram_tensor("w_inT", [128, KCH * 768], bf16, kind="ExternalInput").ap()
    # w_out pre-permuted to [128(d of this core), (n=8, r=4, 512)]
    wout_d = nc.dram_tensor("w_outT", [128, 4 * E], bf16, kind="ExternalInput").ap()
    ropec_d = nc.dram_tensor("rope_c", [128, 640], f32, kind="ExternalInput").ap()
    ropes_d = nc.dram_tensor("rope_s", [128, 640], f32, kind="ExternalInput").ap()
    kT_d = nc.dram_tensor("kT", [B, 128, T], bf16, kind="ExternalInput").ap()
    v_d = nc.dram_tensor("vperm", [B, 128, T], bf16, kind="ExternalInput").ap()
    out_d = nc.dram_tensor("out", [BS, E], f16, kind="ExternalOutput").ap()

    with tile.TileContext(nc) as tc:
        from contextlib import ExitStack

        with ExitStack() as ctx:
            const = ctx.enter_context(tc.tile_pool(name="const", bufs=1))
            winp = ctx.enter_context(tc.tile_pool(name="winp", bufs=1))
            woutp = ctx.enter_context(tc.tile_pool(name="woutp", bufs=1))
            work = ctx.enter_context(tc.tile_pool(name="work", bufs=1))
            kpool = ctx.enter_context(tc.tile_pool(name="kpool", bufs=9))
            vpool = ctx.enter_context(tc.tile_pool(name="vpool", bufs=9))
            epool = ctx.enter_context(tc.tile_pool(name="epool", bufs=4))
            bcp = ctx.enter_context(tc.tile_pool(name="bcp", bufs=2))
            opool = ctx.enter_context(tc.tile_pool(name="opool", bufs=8))
            ps_big = ctx.enter_context(
                tc.tile_pool(name="ps_big", bufs=4, space="PSUM")
            )
            ps_out = ctx.enter_context(
                tc.tile_pool(name="ps_out", bufs=3, space="PSUM")
            )
            ps_sm = ctx.enter_context(tc.tile_pool(name="ps_sm", bufs=1, space="PSUM"))

            # ---- all input DMAs, one hardware queue (sync), priority order.
            # x/w_in pieces interleaved (8 contraction chunks per piece) so
            # the qkv projection starts as soon as piece 0 lands.
            x_p = [
                const.tile([128, 8 * 128], bf16, tag=f"x{j}", name=f"x{j}")
                for j in range(4)
            ]
            w_p = [
                winp.tile([128, 8 * 768], bf16, tag=f"w{j}", name=f"w{j}")
                for j in range(4)
            ]
            ropeC = const.tile([128, 640], f32, tag="ropeC")
            ropeS = const.tile([128, 640], f32, tag="ropeS")
            for j in range(4):
                nc.sync.dma_start(x_p[j][:], xT_d[:, j * 1024 : (j + 1) * 1024])
                nc.sync.dma_start(w_p[j][:], win_d[:, j * 6144 : (j + 1) * 6144])
            # rope tables are needed right after the q/k projection finishes
            nc.sync.dma_start(ropeC[:], ropec_d[:])
            nc.sync.dma_start(ropeS[:], ropes_d[:])

            # kv cache per batch, in halves for finer pipelining; w_out
            # slices dropped in after batch 3 (needed for half-0 out proj)
            kk_h = [[None, None] for _ in range(B)]
            vv_h = [[None, None] for _ in range(B)]
            w_outT = [None, None]
            for b in range(B):
                for h in range(2):
                    kk_h[b][h] = kpool.tile([128, HT], bf16, tag="kk", name=f"kk{b}_{h}")
                    nc.sync.dma_start(kk_h[b][h][:], kT_d[b][:, h * HT : (h + 1) * HT])
                for h in range(2):
                    vv_h[b][h] = vpool.tile([128, HT], bf16, tag="vv", name=f"vv{b}_{h}")
                    nc.sync.dma_start(vv_h[b][h][:], v_d[b][:, h * HT : (h + 1) * HT])
                if b == 3:
                    for h in range(2):
                        w_outT[h] = woutp.tile(
                            [128, 2 * E], bf16, tag=f"wo{h}", name=f"wo{h}"
                        )
                        nc.sync.dma_start(
                            w_outT[h][:], wout_d[:, h * 2 * E : (h + 1) * 2 * E]
                        )

            # ---- constants (vector/gpsimd engines; do not block the DMA queue)
            ident = const.tile([128, 128], f32, tag="ident")
            masks.make_identity(nc, ident[:])
            ones_col = const.tile([128, 1], bf16, tag="ones_col")
            nc.vector.memset(ones_col[:], 1.0)
            ones_row = const.tile([1, 128], f32, tag="ones_row")
            nc.vector.memset(ones_row[:], 1.0)

            # ---- phase 1: q|k projection in psum, piece-paced.
            # q accumulates in a [128, 512] bank, k in a [128, 128] region;
            # the v projection is deferred past the transposes (it only gates
            # the lagged PV stage, not the attention start).
            q_ps = ps_big.tile([128, 512], f32, tag="sc", name="q_ps")
            k_ps = ps_sm.tile([128, 128], f32, tag="sm", name="k_ps")
            for k in range(KCH):
                j, kk_ = k // 8, k % 8
                nc.tensor.matmul(
                    q_ps[:],
                    x_p[j][:, kk_ * 128 : (kk_ + 1) * 128],
                    w_p[j][:, kk_ * 768 : kk_ * 768 + 512],
                    start=(k == 0),
                    stop=(k == KCH - 1),
                )
                nc.tensor.matmul(
                    k_ps[:],
                    x_p[j][:, kk_ * 128 : (kk_ + 1) * 128],
                    w_p[j][:, kk_ * 768 + 512 : kk_ * 768 + 640],
                    start=(k == 0),
                    stop=(k == KCH - 1),
                )

            # ---- rope on q and k: out = t*C + rot(t)*S.  Work is spread
            # across engines to shorten the serial chain: rope-table staging
            # on gpsimd (absorbs the DMA-queue dependency TT ops can't
            # carry), rotate-half copies on scalar, multiplies on vector.
            rot = work.tile([128, 640], f32, tag="rot")
            rot4 = rot[:].rearrange("p (blk h j) -> p blk h j", blk=5, h=2)
            ps4 = q_ps[:].rearrange("p (blk h j) -> p blk h j", blk=4, h=2)
            nc.scalar.copy(rot4[:, 0:4, 0, :], ps4[:, :, 1, :])
            nc.scalar.copy(rot4[:, 0:4, 1, :], ps4[:, :, 0, :])
            nc.scalar.copy(rot[:, 576:640], k_ps[:, 0:64])
            nc.scalar.copy(rot[:, 512:576], k_ps[:, 64:128])
            ropeCs = work.tile([128, 640], f32, tag="ropeCs")
            nc.gpsimd.tensor_copy(ropeCs[:], ropeC[:])
            ropeSs = work.tile([128, 640], f32, tag="ropeSs")
            nc.gpsimd.tensor_copy(ropeSs[:], ropeS[:])
            roped = work.tile([128, 640], f32, tag="roped")
            nc.vector.tensor_mul(roped[:, 0:512], q_ps[:], ropeCs[:, 0:512])
            nc.vector.tensor_mul(roped[:, 512:640], k_ps[:], ropeCs[:, 512:640])
            t2 = work.tile([128, 640], f32, tag="t2")
            nc.vector.tensor_mul(t2[:], rot[:], ropeSs[:])
            nc.vector.tensor_add(roped[:], roped[:], t2[:])

            # ---- transpose q heads -> q_allT [128(d), (b r s)] bf16
            q_allT = work.tile([128, B * R * S], bf16, tag="q_allT")
            qv = q_allT[:].rearrange("p (b r s) -> p b r s", b=B, r=R)
            for r in range(R):
                tp = ps_out.tile([128, 128], f32, tag="po")
                nc.tensor.transpose(tp[:], roped[:, r * 128 : (r + 1) * 128], ident[:])
                nc.vector.tensor_copy(
                    qv[:, :, r, :], tp[:].rearrange("p (b s) -> p b s", b=B)
                )
            # transpose new k -> kT_sb [128(d), (b s)] bf16
            kT_sb = work.tile([128, 128], bf16, tag="kT_sb")
            tpk = ps_out.tile([128, 128], f32, tag="po")
            nc.tensor.transpose(tpk[:], roped[:, 512:640], ident[:])
            nc.vector.tensor_copy(kT_sb[:], tpk[:])

            v_sb = work.tile([128, 128], bf16, tag="v_sb")

            def v_projection():
                # deferred v projection -> v_sb [128(bs), 128(d)] bf16, used
                # to patch the v cache tail per batch (patch is an SBUF->SBUF
                # DMA: compute engines need quadrant-aligned partition starts,
                # DMA does not); emitted after batch-0 scores so it does not
                # delay the attention start (it only gates the lagged PV)
                v_ps = ps_out.tile([128, 128], f32, tag="po", name="v_ps")
                for k in range(KCH):
                    j, kk_ = k // 8, k % 8
                    nc.tensor.matmul(
                        v_ps[:],
                        x_p[j][:, kk_ * 128 : (kk_ + 1) * 128],
                        w_p[j][:, kk_ * 768 + 640 : (kk_ + 1) * 768],
                        start=(k == 0),
                        stop=(k == KCH - 1),
                    )
                nc.vector.tensor_copy(v_sb[:], v_ps[:])

            # attention outputs per half, [128(d), (r, b, s)] bf16: one tile
            # per batch-half so the normalize scale is a single strided TT op
            attn_halfT = [
                work.tile([128, R * 64], bf16, tag=f"attn{h}", name=f"attn{h}")
                for h in range(2)
            ]

            expS = [None] * B
            state = [None] * B  # (outT_ps, recip) per batch

            def emit_out_slice(h, n):
                part_ps = ps_out.tile([64, 512], f32, tag="po", name="part_ps")
                for r in range(R):
                    nc.tensor.matmul(
                        part_ps[:],
                        attn_halfT[h][:, r * 64 : (r + 1) * 64],
                        w_outT[n // 4][
                            :, (n % 4) * 2048 + r * 512 : (n % 4) * 2048 + (r + 1) * 512
                        ],
                        start=(r == 0),
                        stop=(r == R - 1),
                    )
                out_sb = opool.tile([64, 512], f16, tag="out_sb", name="out_sb")
                # half-0 casts must stay off the scalar engine: a cast queued
                # there sits in front of later exp ACTs (program order) and
                # head-of-line-blocks the whole attention pipeline.  In the
                # half-1 (tail) flush all exps are already behind us, so
                # splitting each cast across vector+scalar halves the latency.
                if h == 0:
                    nc.vector.tensor_copy(out_sb[:], part_ps[:])
                else:
                    nc.vector.tensor_copy(out_sb[:, 0:256], part_ps[:, 0:256])
                    nc.scalar.copy(out_sb[:, 256:512], part_ps[:, 256:512])
                nc.gpsimd.dma_start(
                    out_d[h * 64 : (h + 1) * 64, n * 512 : (n + 1) * 512],
                    out_sb[:],
                )

            # ---- phase 2: attention, PV/normalize lagging scores/exp by two
            # batches so the PE stream never waits on the activation engine
            for b in range(B + 2):
                if b < B:
                    # patch stale tail keys/values with roped new ones (the
                    # b==0 v-patch must be issued after v_projection() below:
                    # the tile framework tracks dependencies in program
                    # order, so a read issued before the writer sees garbage)
                    nc.vector.tensor_copy(
                        kk_h[b][1][:, OFFSET - HT : HT], kT_sb[:, b * S : (b + 1) * S]
                    )
                    if b > 0:
                        nc.gpsimd.dma_start(
                            vv_h[b][1][112:128, 15 * 128 : 16 * 128],
                            v_sb[b * S : (b + 1) * S, :],
                        )
                    expS[b] = epool.tile([128, TCH * 64], bf16, tag="expS", name=f"e{b}")
                    # scores in 4 one-bank psum quarters (ring of 4): the PE
                    # can run up to 3 quarters ahead of the exp ACTs instead
                    # of serializing on a 2-deep ring
                    for qtr in range(4):
                        sc = ps_big.tile([128, 512], f32, tag="sc", name=f"sc{b}_{qtr}")
                        for tt in range(8):
                            t = qtr * 8 + tt
                            nc.tensor.matmul(
                                sc[:, tt * 64 : (tt + 1) * 64],
                                kk_h[b][t // 16][:, (t % 16) * 128 : (t % 16 + 1) * 128],
                                q_allT[:, b * 64 : (b + 1) * 64],
                                start=True,
                                stop=True,
                            )
                        nc.scalar.activation(
                            expS[b][:, qtr * 512 : (qtr + 1) * 512],
                            sc[:],
                            ActExp,
                        )
                    if b == 0:
                        v_projection()
                        nc.gpsimd.dma_start(
                            vv_h[0][1][112:128, 15 * 128 : 16 * 128],
                            v_sb[0:S, :],
                        )
                if b > 1:
                    pb = b - 2
                    # denom first: its reciprocal then computes on vector
                    # while the PE streams PV, instead of serializing after
                    denom_ps = ps_sm.tile([1, 64], f32, tag="sm", name=f"d{pb}")
                    for t in range(TCH):
                        nc.tensor.matmul(
                            denom_ps[:],
                            ones_col[:],
                            expS[pb][:, t * 64 : (t + 1) * 64],
                            start=(t == 0),
                            stop=(t == TCH - 1),
                        )
                    recip = bcp.tile([1, 64], f32, tag="recip", name=f"r{pb}")
                    nc.vector.reciprocal(recip[:], denom_ps[:])
                    outT_ps = ps_out.tile([128, 64], f32, tag="po", name=f"o{pb}")
                    for t in range(TCH):
                        nc.tensor.matmul(
                            outT_ps[:],
                            vv_h[pb][t // 16][:, (t % 16) * 128 : (t % 16 + 1) * 128],
                            expS[pb][:, t * 64 : (t + 1) * 64],
                            start=(t == 0),
                            stop=(t == TCH - 1),
                        )
                    bc_ps = ps_sm.tile([128, 64], f32, tag="sm", name=f"bc{pb}")
                    nc.tensor.matmul(
                        bc_ps[:], ones_row[:], recip[:], start=True, stop=True
                    )
                    bc_sb = bcp.tile([128, 64], f32, tag="bc_sb", name=f"bc_sb{pb}")
                    nc.vector.tensor_copy(bc_sb[:], bc_ps[:])
                    # single strided scale: attn[(r, pb%4, s)] = outT[(r, s)] * bc[(r, s)]
                    attn_view = attn_halfT[pb // 4][:].rearrange(
                        "p (r b s) -> p r b s", r=R, b=4
                    )
                    nc.vector.tensor_mul(
                        attn_view[:, :, pb % 4, :],
                        outT_ps[:].rearrange("p (r s) -> p r s", r=R),
                        bc_sb[:].rearrange("p (r s) -> p r s", r=R),
                    )
                    # spread half-0's out-proj two slices per batch across
                    # batches 4-7 so it never blocks the scores/PV pipeline;
                    # half-1's slices can only run at the very end
                    if 3 <= pb <= 6:
                        emit_out_slice(0, 2 * (pb - 3))
                        emit_out_slice(0, 2 * (pb - 3) + 1)
                    elif pb == 7:
                        for n in range(8):
                            emit_out_slice(1, n)

    nc.compile()
    return nc


def _host_shards(x, w_in, w_out, k_cache, v_cache):
    """Per-core input dicts, pre-permuted for contiguous [128, N] DMAs."""
    x2 = np.ascontiguousarray(x.reshape(BS, E))
    xT_perm = (
        x2.T.reshape(KCH, 128, 128).transpose(1, 0, 2).reshape(128, KCH * 128)
    ).astype(BF16)

    # rope tables (identical on all cores); fold attn scale into q blocks
    inv_freq = 1.0 / (ROPE_BASE ** (np.arange(0, HD, 2, dtype=np.float64) / HD))
    pos = (OFFSET + np.arange(S)).astype(np.float64)
    ang = pos[:, None] * inv_freq[None, :]          # [S, 64]
    cos16 = np.cos(ang).astype(np.float32)
    sin16 = np.sin(ang).astype(np.float32)
    scale = np.float32(1.0 / np.sqrt(HD))
    C = np.zeros((128, 640), np.float32)
    Sn = np.zeros((128, 640), np.float32)
    srow = np.arange(128) % S                        # partition p=(b,s) -> s
    for blk in range(5):
        blk_scale = scale if blk < 4 else np.float32(1.0)
        C[:, blk * 128 : blk * 128 + 64] = cos16[srow] * blk_scale
        C[:, blk * 128 + 64 : blk * 128 + 128] = cos16[srow] * blk_scale
        Sn[:, blk * 128 : blk * 128 + 64] = -sin16[srow] * blk_scale
        Sn[:, blk * 128 + 64 : blk * 128 + 128] = sin16[srow] * blk_scale

    shards = []
    for g in range(NCORES):
        rows = np.concatenate(
            [
                w_in[QF * g : QF * (g + 1)],
                w_in[E + HD * g : E + HD * (g + 1)],
                w_in[E + HKV * HD + HD * g : E + HKV * HD + HD * (g + 1)],
            ],
            axis=0,
        )  # [768, 4096]
        w_inT_perm = (
            rows.T.reshape(KCH, 128, 768)
            .transpose(1, 0, 2)
            .reshape(128, KCH * 768)
        ).astype(BF16)
        # [128(d), (r, e)] -> [128(d), (n, r, 512)] so out-proj slice n is
        # contiguous and w_out can stream in two ordered pieces
        w_outT_perm = (
            w_out[:, QF * g : QF * (g + 1)]
            .T.reshape(4, 128, E)
            .transpose(1, 0, 2)
            .reshape(128, 4, 8, 512)
            .transpose(0, 2, 1, 3)
            .reshape(128, 4 * E)
        ).astype(BF16)
        kT = np.ascontiguousarray(
            k_cache[:, :, g, :].transpose(0, 2, 1)
        ).astype(BF16)  # [B, 128(d), T]
        vperm = np.ascontiguousarray(
            v_cache[:, :, g, :]
            .reshape(B, TCH, 128, HD)
            .transpose(0, 2, 1, 3)
            .reshape(B, 128, T)
        ).astype(BF16)  # [B, 128(t_in), (chunk d)]
        shards.append(
            {
                "xT": xT_perm,
                "w_inT": w_inT_perm,
                "w_outT": w_outT_perm,
                "rope_c": C,
                "rope_s": Sn,
                "kT": kT,
                "vperm": vperm,
            }
        )
    return shards


def _get_nc():
    if "nc" not in _CACHED:
        _CACHED["nc"] = _build_program()
    return _CACHED["nc"]


def run_on_hw(in_maps, trace=False, **kw):
    from concourse import bass_utils

    nc = _get_nc()
    return bass_utils.run_bass_kernel_spmd(
        nc, in_maps, core_ids=list(range(NCORES)), trace=trace, **kw
    )


def kernel(x, w_in, w_out, k_cache, v_cache, offset):
    assert int(offset) == OFFSET and x.shape == (B, S, E)
    shards = _host_shards(
        np.asarray(x, np.float32),
        np.asarray(w_in, np.float32),
        np.asarray(w_out, np.float32),
        np.asarray(k_cache, np.float32),
        np.asarray(v_cache, np.float32),
    )
    res = run_on_hw(shards)
    out = np.zeros((BS, E), np.float64)
    for g in range(NCORES):
        out += np.asarray(res.results[g]["out"], np.float64)
    return out.astype(np.float32).reshape(B, S, E)
